# revision 1
# baseline (speedup 1.0000x reference)
"""Mamba block kernel for 8 Trainium2 NeuronCores (Bass/Tile, SPMD).

Sharding: 2-way data-parallel over batch x 4-way tensor-parallel over
d_inner. Core c handles batch c//4 and channel block c%4 (512 channels).

Per-core pipeline (activations kept channel-major [d, t] on chip):
  PE-transpose hs -> in_proj (fp32r matmuls) -> depthwise conv + SiLU ->
  x_dbl partial + 4-core AllReduce -> delta = softplus(dt proj) ->
  selective scan over two L/2 halves: per state-dim n: B_n/C_n row
  broadcast via one stride-0 DMA, dA = exp(delta*A_n) on ScalarE,
  dBu = delta*u*B_n on GpSimd, native DVE tensor_tensor_scan recurrence
  (carry chained between halves), y = sum_n C_n*s_n + D*u accumulated in
  PSUM via identity/diag matmuls on TensorE -> gate with z*sigmoid(z)
  (tanh form, 0.5 factor folded into the out_proj weights host-side) ->
  out_proj partial -> 4-core ReduceScatter.
"""
import numpy as np

BATCH, L, D_MODEL = 2, 2048, 1024
D_INNER, D_STATE, D_CONV, DT_RANK = 2048, 16, 4, 64
NCORES = 8
DC = D_INNER // 4          # 512 channels per core
NBLK = DC // 128           # 4 partition blocks
P = 128
NQ = L // 512              # 4 time quarters for matmul N-tiling
H = L // 2                 # scan half length

TRACE = False              # set by test.py to capture a profile
LAST_RESULTS = None        # BassKernelResults stash for test.py

_prog_cache = {}


class _Stop(Exception):
    pass


def _build_program(collectives=True, upto="F"):
    lvl = {"A": 1, "B": 2, "D": 3, "E": 4, "F": 5}[upto]
    import concourse.bass as bass
    import concourse.mybir as mybir
    import concourse.tile as tile
    from concourse import bacc
    from concourse.masks import make_identity
    from contextlib import ExitStack

    f32 = mybir.dt.float32
    f32r = mybir.dt.float32r
    MULT = mybir.AluOpType.mult
    ADD = mybir.AluOpType.add
    ACT = mybir.ActivationFunctionType

    nc = bacc.Bacc("TRN2", target_bir_lowering=False, debug=False,
                   num_devices=NCORES)

    # ---- kernel I/O (per-core shapes; host pre-arranges layouts) ----
    hsb = nc.dram_tensor("hsb", [L, D_MODEL], f32, kind="ExternalInput")
    wxz = nc.dram_tensor("wxz", [P, 8, 1024], f32, kind="ExternalInput")
    convw = nc.dram_tensor("convw", [P, NBLK * D_CONV], f32, kind="ExternalInput")
    convb = nc.dram_tensor("convb", [P, NBLK], f32, kind="ExternalInput")
    xprojT = nc.dram_tensor("xprojT", [P, NBLK, 96], f32, kind="ExternalInput")
    dtprojT = nc.dram_tensor("dtprojT", [DT_RANK, DC], f32, kind="ExternalInput")
    dtb = nc.dram_tensor("dtb", [P, NBLK], f32, kind="ExternalInput")
    alog = nc.dram_tensor("alog", [P, NBLK * D_STATE], f32, kind="ExternalInput")
    dvec = nc.dram_tensor("dvec", [P, NBLK], f32, kind="ExternalInput")
    outw = nc.dram_tensor("outw", [P, NBLK, 1024], f32, kind="ExternalInput")

    out_shard = nc.dram_tensor("out_shard", [D_MODEL // 4, L], f32,
                               kind="ExternalOutput")

    # ---- internal DRAM ----
    xdbl_part = nc.dram_tensor("xdbl_part", [96, L], f32)
    xdbl_sum = nc.dram_tensor("xdbl_sum", [96, L], f32)
    uspill = nc.dram_tensor("uspill", [DC, L], f32r)
    outpart = nc.dram_tensor("outpart", [D_MODEL, L], f32)
    outrs = nc.dram_tensor("outrs", [D_MODEL // 4, L], f32)

    GROUPS = [[0, 1, 2, 3], [4, 5, 6, 7]]

    with tile.TileContext(nc) as tc, ExitStack() as top:
        const_pool = top.enter_context(tc.tile_pool(name="const", bufs=1))
        # z is kept resident through the gate; allocate below const on the
        # left stack so every other left pool nests inside its lifetime.
        z_pool = top.enter_context(tc.tile_pool(name="zpool", bufs=1))

        ident = const_pool.tile([P, P], f32)
        make_identity(nc, ident)
        ident_r = const_pool.tile([P, P], f32r)
        nc.scalar.copy(ident_r[:], ident[:])
        ones_col = const_pool.tile([P, 1], f32)
        nc.vector.memset(ones_col[:], 1.0)

        convw_sb = const_pool.tile([P, NBLK * D_CONV], f32)
        nc.sync.dma_start(convw_sb[:], convw[:])
        convb_sb = const_pool.tile([P, NBLK], f32)
        nc.sync.dma_start(convb_sb[:], convb[:])
        dtb_sb = const_pool.tile([P, NBLK], f32)
        nc.sync.dma_start(dtb_sb[:], dtb[:])
        dvec_sb = const_pool.tile([P, NBLK], f32)
        nc.sync.dma_start(dvec_sb[:], dvec[:])
        alog_sb = const_pool.tile([P, NBLK * D_STATE], f32)
        nc.sync.dma_start(alog_sb[:], alog[:])
        negA = const_pool.tile([P, NBLK * D_STATE], f32)
        nc.scalar.activation(negA[:], alog_sb[:], ACT.Exp)
        nc.scalar.mul(negA[:], negA[:], -1.0)
        # diag(D) per block, fp32r, for the y-psum init matmul
        diagD = const_pool.tile([P, NBLK, P], f32r)
        for blk in range(NBLK):
            dtmp = const_pool.tile([P, P], f32, tag="dtmp", name="dtmp")
            nc.vector.tensor_scalar_mul(dtmp[:], ident[:],
                                        dvec_sb[:, blk:blk + 1])
            nc.scalar.copy(diagD[:, blk, :], dtmp[:])

        z_sb = z_pool.tile([P, NBLK, L], f32)

        es_A = ExitStack()    # hsT + in_proj weights, spans T+A
        es_xz = ExitStack()   # conv input tiles, spans A..B
        es_u = ExitStack()    # u tiles, spans B..D
        es_wc = ExitStack()   # x_dbl/dt_proj weights, spans C..D
        es_ddu = ExitStack()  # delta+du, spans D..E
        es_E = ExitStack()    # scan transient pools
        es_yg = ExitStack()   # gated y, spans E..F
        es_F = ExitStack()    # out_proj pools
        es_psA = ExitStack()  # matmul psum pools, spans T..D

        try:
            # ===== Phase T+A: transpose hs / in_proj, per time quarter =====
            hsT_pool = es_A.enter_context(tc.tile_pool(name="hsTpool", bufs=1))
            ps_t = es_psA.enter_context(
                tc.tile_pool(name="ps_t", bufs=3, space="PSUM"))
            ps_a = es_psA.enter_context(
                tc.tile_pool(name="ps_a", bufs=4, space="PSUM"))

            hsT = hsT_pool.tile([P, 8, L], f32r, tag="hsT")
            w_r = hsT_pool.tile([P, 8, 1024], f32r, tag="wr")
            with tc.tile_pool(name="wstage", bufs=2) as w_stage:
                for kb in range(8):
                    w_sb = w_stage.tile([P, 1024], f32, tag="w", name=f"w{kb}")
                    nc.sync.dma_start(w_sb[:], wxz[:, kb, :])
                    nc.scalar.copy(w_r[:, kb, :], w_sb[:])

            xz_pool = es_xz.enter_context(
                tc.tile_pool(name="xzpool", bufs=1, side="right"))
            xp = [xz_pool.tile([P, 4 + L], f32, tag=f"xp{b}", name=f"xp{b}")
                  for b in range(NBLK)]
            for blk in range(NBLK):
                nc.vector.memset(xp[blk][:, 0:4], 0.0)

            with tc.tile_pool(name="hsbpool", bufs=2) as hsb_pool:
                for tq in range(NQ):
                    # transpose the 4 token-blocks of this quarter
                    hsb_sb = hsb_pool.tile([P, 4, D_MODEL], f32, tag="hsb",
                                           name=f"hsb{tq}")
                    nc.sync.dma_start(
                        hsb_sb[:],
                        hsb[tq * 512:(tq + 1) * 512, :]
                        .rearrange("(tb p) m -> p tb m", p=P))
                    for kb in range(8):
                        pst = ps_t.tile([P, 512], f32, tag="pst", name="pst")
                        for tb in range(4):
                            nc.tensor.transpose(
                                pst[:, tb * P:(tb + 1) * P],
                                hsb_sb[:, tb, kb * P:(kb + 1) * P],
                                ident[:])
                        nc.vector.tensor_copy(
                            hsT[:, kb, tq * 512:(tq + 1) * 512], pst[:])
                    # in_proj for this quarter
                    for mb in range(8):
                        ps = ps_a.tile([P, 512], f32, tag="psa", name="psa")
                        for kb in range(8):
                            nc.tensor.matmul(
                                ps[:], w_r[:, kb, mb * P:(mb + 1) * P],
                                hsT[:, kb, tq * 512:(tq + 1) * 512],
                                start=(kb == 0), stop=(kb == 7))
                        if mb < 4:
                            nc.scalar.copy(
                                xp[mb][:, 4 + tq * 512: 4 + (tq + 1) * 512],
                                ps[:])
                        else:
                            nc.scalar.copy(
                                z_sb[:, mb - 4, tq * 512:(tq + 1) * 512],
                                ps[:])
            es_A.close()
            if lvl < 2:
                raise _Stop()

            # ============ Phase B: conv + SiLU ============
            u_pool = es_u.enter_context(tc.tile_pool(name="upool", bufs=1))
            u_r = u_pool.tile([P, NBLK, L], f32r)
            with tc.tile_pool(name="cvpool", bufs=2) as cv_pool:
                for blk in range(NBLK):
                    acc = cv_pool.tile([P, L], f32, tag="acc", name="acc")
                    nc.vector.tensor_scalar_mul(
                        acc[:], xp[blk][:, 4:4 + L],
                        convw_sb[:, blk * 4 + 3: blk * 4 + 4])
                    for w in (2, 1, 0):
                        nc.vector.scalar_tensor_tensor(
                            acc[:], xp[blk][:, 1 + w: 1 + w + L],
                            convw_sb[:, blk * 4 + w: blk * 4 + w + 1],
                            acc[:], MULT, ADD)
                    nc.scalar.activation(u_r[:, blk, :], acc[:], ACT.Silu,
                                         bias=convb_sb[:, blk:blk + 1])
            es_xz.close()
            if lvl < 3:
                raise _Stop()

            # ============ Phase C: x_dbl partial + AllReduce ============
            wc_pool = es_wc.enter_context(tc.tile_pool(name="wcpool", bufs=1))
            xw_sb = wc_pool.tile([P, NBLK, 96], f32, tag="xw")
            nc.sync.dma_start(xw_sb[:], xprojT[:])
            xw_r = wc_pool.tile([P, NBLK, 96], f32r, tag="xwr")
            nc.scalar.copy(xw_r[:], xw_sb[:])
            with tc.tile_pool(name="xepool", bufs=3) as xe_pool:
                for tq in range(NQ):
                    ps = ps_a.tile([P, 512], f32, tag="psa", name="psa")
                    for kb in range(NBLK):
                        nc.tensor.matmul(ps[0:96, :], xw_r[:, kb, :],
                                         u_r[:, kb, tq * 512:(tq + 1) * 512],
                                         start=(kb == 0), stop=(kb == 3))
                    xe = xe_pool.tile([P, 512], f32, tag="xe", name="xe")
                    nc.scalar.copy(xe[0:96, :], ps[0:96, :])
                    nc.sync.dma_start(
                        xdbl_part[:, tq * 512:(tq + 1) * 512], xe[0:96, :])

            if collectives:
                nc.gpsimd.collective_compute(
                    "AllReduce", ADD, replica_groups=GROUPS,
                    ins=[xdbl_part[:]], outs=[xdbl_sum[:]])
            else:  # timing-sim variant: plain copy stands in for the AR
                nc.sync.dma_start(xdbl_sum[:], xdbl_part[:])

            # ============ Phase D: delta = softplus(dt_proj), du ============
            dtr_sb = wc_pool.tile([DT_RANK, L], f32, tag="dtr")
            nc.sync.dma_start(dtr_sb[:], xdbl_sum[0:DT_RANK, :])
            dtr_r = wc_pool.tile([DT_RANK, L], f32r, tag="dtrr")
            nc.scalar.copy(dtr_r[:], dtr_sb[:])
            dtw_sb = wc_pool.tile([DT_RANK, DC], f32, tag="dtw")
            nc.sync.dma_start(dtw_sb[:], dtprojT[:])
            dtw_r = wc_pool.tile([DT_RANK, DC], f32r, tag="dtwr")
            nc.scalar.copy(dtw_r[:], dtw_sb[:])

            ddu_pool = es_ddu.enter_context(
                tc.tile_pool(name="ddupool", bufs=1, side="right"))
            delta = ddu_pool.tile([P, NBLK, L], f32)
            du = ddu_pool.tile([P, NBLK, L], f32)
            # softplus(x+b) = relu(x+b) + ln(1 + exp(-|x+b|)); the deployed
            # Softplus table is unavailable, so compose it. Batch the Exp
            # and Ln sub-steps per block so the Exp<->Ln table sets load
            # once per batch instead of per chunk.
            with tc.tile_pool(name="sppool", bufs=1) as sp_pool:
                for blk in range(NBLK):
                    pss, tes = [], []
                    for tq in range(NQ):
                        ps = ps_a.tile([P, 512], f32, tag="psa", name="psa")
                        nc.tensor.matmul(
                            ps[:], dtw_r[:, blk * P:(blk + 1) * P],
                            dtr_r[:, tq * 512:(tq + 1) * 512])
                        pss.append(ps)
                        ta = sp_pool.tile([P, 512], f32, tag="ta", name="ta")
                        nc.scalar.activation(ta[:], ps[:], ACT.Abs,
                                             bias=dtb_sb[:, blk:blk + 1])
                        te = sp_pool.tile([P, 512], f32, tag=f"te{tq}",
                                          name="te")
                        nc.scalar.activation(te[:], ta[:], ACT.Exp, scale=-1.0)
                        tes.append(te)
                    for tq in range(NQ):
                        dchunk = delta[:, blk, tq * 512:(tq + 1) * 512]
                        tl = sp_pool.tile([P, 512], f32, tag="tl", name="tl")
                        nc.scalar.activation(tl[:], tes[tq][:], ACT.Ln,
                                             bias=1.0)
                        tr = sp_pool.tile([P, 512], f32, tag="tr", name="tr")
                        nc.scalar.activation(tr[:], pss[tq][:], ACT.Relu,
                                             bias=dtb_sb[:, blk:blk + 1])
                        nc.vector.tensor_tensor(dchunk, tr[:], tl[:],
                                                ADD)
                    nc.vector.tensor_tensor(du[:, blk, :], delta[:, blk, :],
                                            u_r[:, blk, :], MULT)
                    nc.sync.dma_start(uspill[blk * P:(blk + 1) * P, :],
                                      u_r[:, blk, :])
            es_wc.close()
            es_u.close()
            es_psA.close()
            if lvl < 4:
                raise _Stop()

            # ============ Phase E: selective scan (two halves) ============
            yg_pool = es_yg.enter_context(tc.tile_pool(name="ygpool", bufs=1))
            bc_pool = es_E.enter_context(tc.tile_pool(name="bcpool", bufs=2))
            sc_pool = es_E.enter_context(tc.tile_pool(name="scpool", bufs=3))
            s_pool = es_E.enter_context(tc.tile_pool(name="spool", bufs=2))
            g_pool = es_E.enter_context(tc.tile_pool(name="gpool", bufs=1))
            carry_pool = es_E.enter_context(tc.tile_pool(name="carrypool",
                                                         bufs=1))
            ps_y = es_E.enter_context(
                tc.tile_pool(name="ps_y", bufs=4, space="PSUM"))
            yg = yg_pool.tile([P, NBLK, L], f32r)
            carry = carry_pool.tile([P, NBLK * D_STATE], f32)

            for half in range(2):
                h0 = half * H
                y_ps = {}
                for blk in range(NBLK):
                    y_ps[blk] = ps_y.tile([P, H], f32, tag="yps",
                                          name=f"yps{blk}")
                    uc = g_pool.tile([P, H], f32r, tag="uc", name="uc")
                    nc.sync.dma_start(
                        uc[:], uspill[blk * P:(blk + 1) * P, h0:h0 + H])
                    for c in range(2):
                        nc.tensor.matmul(
                            y_ps[blk][:, c * 512:(c + 1) * 512],
                            diagD[:, blk, :],
                            uc[:, c * 512:(c + 1) * 512],
                            start=True, stop=False)
                for n in range(D_STATE):
                    # B_n and C_n rows are interleaved host-side: one
                    # stride-0 partition-broadcast DMA fetches both.
                    bc = bc_pool.tile([P, 2, H], f32, tag="bc", name="bc")
                    nc.sync.dma_start(
                        bc[:],
                        xdbl_sum[DT_RANK + 2 * n: DT_RANK + 2 * n + 2,
                                 h0:h0 + H]
                        .partition_broadcast(P))
                    for blk in range(NBLK):
                        idx = blk * D_STATE + n
                        dchunk = delta[:, blk, h0:h0 + H]
                        dA = sc_pool.tile([P, H], f32, tag="dA", name="dA")
                        nc.scalar.activation(
                            dA[:], dchunk, ACT.Exp,
                            scale=negA[:, idx:idx + 1])
                        dBu = sc_pool.tile([P, H], f32, tag="dBu", name="dBu")
                        nc.gpsimd.tensor_tensor(dBu[:], du[:, blk, h0:h0 + H],
                                                bc[:, 0, :], MULT)
                        s = s_pool.tile([P, H], f32, tag="s", name="s")
                        nc.vector.tensor_tensor_scan(
                            s[:], dA[:], dBu[:],
                            0.0 if half == 0 else carry[:, idx:idx + 1],
                            MULT, ADD)
                        if half == 0:
                            nc.scalar.copy(carry[:, idx:idx + 1],
                                           s[:, H - 1:H])
                        sC = s_pool.tile([P, H], f32r, tag="sC", name="sC")
                        # balance the C-multiply across DVE and GpSimd
                        if idx % 16 == 5:
                            nc.gpsimd.tensor_tensor(sC[:], s[:], bc[:, 1, :],
                                                    MULT)
                        else:
                            nc.vector.tensor_tensor(sC[:], s[:], bc[:, 1, :],
                                                    MULT)
                        for c in range(2):
                            nc.tensor.matmul(
                                y_ps[blk][:, c * 512:(c + 1) * 512],
                                ident_r[:],
                                sC[:, c * 512:(c + 1) * 512],
                                start=False, stop=(n == D_STATE - 1))
                # gate: yg = y * z * (tanh(z/2) + 1); the 0.5 factor is
                # folded into the out_proj weights host-side.
                for blk in range(NBLK):
                    zchunk = z_sb[:, blk, h0:h0 + H]
                    th = g_pool.tile([P, H], f32, tag="th", name="th")
                    nc.scalar.activation(th[:], zchunk, ACT.Tanh, scale=0.5)
                    t1 = g_pool.tile([P, H], f32, tag="t1", name="t1")
                    nc.vector.scalar_tensor_tensor(
                        t1[:], th[:], ones_col[:], zchunk, ADD, MULT)
                    nc.vector.tensor_tensor(
                        yg[:, blk, h0:h0 + H], y_ps[blk][:], t1[:], MULT)
            es_ddu.close()
            es_E.close()
            if lvl < 5:
                raise _Stop()

            # ===== Phase F: out_proj partial + ReduceScatter =====
            ow_pool = es_F.enter_context(tc.tile_pool(name="owpool", bufs=1))
            oe_pool = es_F.enter_context(tc.tile_pool(name="oepool", bufs=3))
            ps_o = es_F.enter_context(
                tc.tile_pool(name="ps_o", bufs=4, space="PSUM"))

            ow_r = ow_pool.tile([P, NBLK, 1024], f32r, tag="owr")
            with tc.tile_pool(name="owstage", bufs=2) as ow_stage:
                for kb in range(NBLK):
                    ow_sb = ow_stage.tile([P, 1024], f32, tag="ow",
                                          name=f"ow{kb}")
                    nc.sync.dma_start(ow_sb[:], outw[:, kb, :])
                    nc.scalar.copy(ow_r[:, kb, :], ow_sb[:])

            for mb in range(8):
                ot = oe_pool.tile([P, L], f32, tag="ot", name="ot")
                for tq in range(NQ):
                    po = ps_o.tile([P, 512], f32, tag="po", name="po")
                    for kb in range(NBLK):
                        nc.tensor.matmul(
                            po[:], ow_r[:, kb, mb * P:(mb + 1) * P],
                            yg[:, kb, tq * 512:(tq + 1) * 512],
                            start=(kb == 0), stop=(kb == 3))
                    nc.scalar.copy(ot[:, tq * 512:(tq + 1) * 512], po[:])
                nc.sync.dma_start(outpart[mb * P:(mb + 1) * P, :], ot[:])

            if collectives:
                nc.gpsimd.collective_compute(
                    "ReduceScatter", ADD, replica_groups=GROUPS,
                    ins=[outpart[:]], outs=[outrs[:]])
            else:
                nc.sync.dma_start(outrs[:], outpart[0:D_MODEL // 4, :])
            nc.sync.dma_start(out_shard[:], outrs[:])
            es_F.close()
            es_yg.close()
        except _Stop:
            pass
        finally:
            for es in (es_A, es_xz, es_u, es_wc, es_ddu, es_E,
                       es_F, es_yg, es_psA):
                es.close()

    nc.compile()
    return nc


def _shard_inputs(inputs):
    hs = np.ascontiguousarray(np.asarray(inputs["hidden_states"], np.float32))
    win = np.asarray(inputs["in_proj_w"], np.float32)
    convw = np.asarray(inputs["conv_w"], np.float32)
    convb = np.asarray(inputs["conv_b"], np.float32)
    xproj = np.asarray(inputs["x_proj_w"], np.float32)
    dtw = np.asarray(inputs["dt_proj_w"], np.float32)
    dtb = np.asarray(inputs["dt_proj_b"], np.float32)
    alog = np.asarray(inputs["A_log"], np.float32)
    dv = np.asarray(inputs["D"], np.float32)
    outw = np.asarray(inputs["out_proj_w"], np.float32)

    # permute x_proj rows so B_n/C_n come out interleaved: one broadcast DMA
    # per state-dim fetches both rows.
    perm = list(range(DT_RANK))
    for n in range(D_STATE):
        perm.append(DT_RANK + n)            # B_n -> row 64+2n
        perm.append(DT_RANK + D_STATE + n)  # C_n -> row 65+2n
    xproj_p = xproj[perm]

    in_maps = []
    for c in range(NCORES):
        b, k = c // 4, c % 4
        cs, ce = k * DC, (k + 1) * DC
        wxT = win[cs:ce].T            # [1024, 512]
        wzT = win[D_INNER + cs:D_INNER + ce].T
        wcat = np.concatenate([wxT, wzT], axis=1)  # [1024, 1024]
        wxz = np.ascontiguousarray(
            wcat.reshape(8, P, 1024).transpose(1, 0, 2))  # [128, 8, 1024]
        in_maps.append({
            "hsb": hs[b],
            "wxz": wxz,
            "convw": np.ascontiguousarray(
                convw[cs:ce].reshape(NBLK, P, D_CONV).transpose(1, 0, 2)
                .reshape(P, NBLK * D_CONV)),
            "convb": np.ascontiguousarray(convb[cs:ce].reshape(NBLK, P).T),
            "xprojT": np.ascontiguousarray(
                xproj_p[:, cs:ce].T.reshape(NBLK, P, 96).transpose(1, 0, 2)),
            "dtprojT": np.ascontiguousarray(dtw[cs:ce].T),
            "dtb": np.ascontiguousarray(dtb[cs:ce].reshape(NBLK, P).T),
            "alog": np.ascontiguousarray(
                alog[cs:ce].reshape(NBLK, P, D_STATE).transpose(1, 0, 2)
                .reshape(P, NBLK * D_STATE)),
            "dvec": np.ascontiguousarray(dv[cs:ce].reshape(NBLK, P).T),
            # 0.5 * W_out^T: the tanh-form gate leaves a factor 2
            "outw": np.ascontiguousarray(
                (0.5 * outw[:, cs:ce]).T.reshape(NBLK, P, 1024)
                .transpose(1, 0, 2)),
        })
    return in_maps


def kernel(**inputs):
    global LAST_RESULTS
    from concourse.bass_utils import run_bass_kernel_spmd

    if "prog" not in _prog_cache:
        _prog_cache["prog"] = _build_program()
    nc = _prog_cache["prog"]

    in_maps = _shard_inputs(inputs)
    res = run_bass_kernel_spmd(nc, in_maps, list(range(NCORES)),
                               trace=TRACE)
    LAST_RESULTS = res

    out = np.empty((BATCH, L, D_MODEL), np.float32)
    for g in range(BATCH):
        rows = np.concatenate(
            [res.results[g * 4 + i]["out_shard"] for i in range(4)], axis=0)
        out[g] = rows.T
    return out



# revision 12
# speedup vs baseline: 1.0572x; 1.0572x over previous
"""Mamba block kernel for 8 Trainium2 NeuronCores (Bass/Tile, SPMD).

Sharding: 2-way data-parallel over batch x 4-way tensor-parallel over
d_inner. Core c handles batch c//4 and channel block c%4 (512 channels).

Per-core pipeline (activations kept channel-major [d, t] on chip):
  PE-transpose hs -> in_proj (fp32r matmuls) -> depthwise conv + SiLU ->
  x_dbl partial + 4-core AllReduce -> delta = softplus(dt proj) ->
  selective scan over two L/2 halves: per state-dim n: B_n/C_n row
  broadcast via one stride-0 DMA, dA = exp(delta*A_n) on ScalarE,
  dBu = delta*u*B_n on GpSimd, native DVE tensor_tensor_scan recurrence
  (carry chained between halves), y = sum_n C_n*s_n + D*u accumulated in
  PSUM via identity/diag matmuls on TensorE -> gate with z*sigmoid(z)
  (tanh form, 0.5 factor folded into the out_proj weights host-side) ->
  out_proj partial -> 4-core ReduceScatter.
"""
import numpy as np

BATCH, L, D_MODEL = 2, 2048, 1024
D_INNER, D_STATE, D_CONV, DT_RANK = 2048, 16, 4, 64
NCORES = 8
DC = D_INNER // 4          # 512 channels per core
NBLK = DC // 128           # 4 partition blocks
P = 128
NQ = L // 512              # 4 time quarters for matmul N-tiling
H = L // 2                 # scan half length

TRACE = False              # set by test.py to capture a profile
LAST_RESULTS = None        # BassKernelResults stash for test.py

_prog_cache = {}


class _Stop(Exception):
    pass


def _build_program(collectives=True, upto="F"):
    lvl = {"A": 1, "B": 2, "D": 3, "E": 4, "F": 5}[upto]
    import concourse.bass as bass
    import concourse.mybir as mybir
    import concourse.tile as tile
    from concourse import bacc
    from concourse.masks import make_identity
    from contextlib import ExitStack

    f32 = mybir.dt.float32
    f32r = mybir.dt.float32r
    bf16 = mybir.dt.bfloat16
    MULT = mybir.AluOpType.mult
    ADD = mybir.AluOpType.add
    ACT = mybir.ActivationFunctionType

    nc = bacc.Bacc("TRN2", target_bir_lowering=False, debug=False,
                   num_devices=NCORES)

    # ---- kernel I/O (per-core shapes; host pre-arranges layouts) ----
    hsb = nc.dram_tensor("hsb", [L, D_MODEL], f32, kind="ExternalInput")
    wxz = nc.dram_tensor("wxz", [P, 8, 1024], f32, kind="ExternalInput")
    convw = nc.dram_tensor("convw", [P, NBLK * D_CONV], f32, kind="ExternalInput")
    convb = nc.dram_tensor("convb", [P, NBLK], f32, kind="ExternalInput")
    xprojT = nc.dram_tensor("xprojT", [P, NBLK, 96], f32, kind="ExternalInput")
    dtprojT = nc.dram_tensor("dtprojT", [DT_RANK, DC], f32, kind="ExternalInput")
    dtb = nc.dram_tensor("dtb", [P, NBLK], f32, kind="ExternalInput")
    alog = nc.dram_tensor("alog", [P, NBLK * D_STATE], f32, kind="ExternalInput")
    dvec = nc.dram_tensor("dvec", [P, NBLK], f32, kind="ExternalInput")
    outw = nc.dram_tensor("outw", [P, NBLK, 1024], f32, kind="ExternalInput")

    out_shard = nc.dram_tensor("out_shard", [D_MODEL // 4, L], f32,
                               kind="ExternalOutput")

    # ---- internal DRAM ----
    xdbl_part = nc.dram_tensor("xdbl_part", [96, L], bf16)
    xdbl_sum = nc.dram_tensor("xdbl_sum", [96, L], bf16)
    uspill = nc.dram_tensor("uspill", [DC, L], bf16)
    outpart = nc.dram_tensor("outpart", [D_MODEL, L], f32)
    outrs = nc.dram_tensor("outrs", [D_MODEL // 4, L], f32)

    GROUPS = [[0, 1, 2, 3], [4, 5, 6, 7]]

    with tile.TileContext(nc) as tc, ExitStack() as top:
        const_pool = top.enter_context(tc.tile_pool(name="const", bufs=1))
        # z is kept resident through the gate; allocate below const on the
        # left stack so every other left pool nests inside its lifetime.
        z_pool = top.enter_context(tc.tile_pool(name="zpool", bufs=1))

        ident = const_pool.tile([P, P], f32)
        make_identity(nc, ident)
        ident16 = const_pool.tile([P, P], bf16)
        nc.scalar.copy(ident16[:], ident[:])
        ones_col = const_pool.tile([P, 1], f32)
        nc.vector.memset(ones_col[:], 1.0)

        convw_sb = const_pool.tile([P, NBLK * D_CONV], f32)
        nc.sync.dma_start(convw_sb[:], convw[:])
        convb_sb = const_pool.tile([P, NBLK], f32)
        nc.sync.dma_start(convb_sb[:], convb[:])
        dtb_sb = const_pool.tile([P, NBLK], f32)
        nc.sync.dma_start(dtb_sb[:], dtb[:])
        dvec_sb = const_pool.tile([P, NBLK], f32)
        nc.sync.dma_start(dvec_sb[:], dvec[:])
        alog_sb = const_pool.tile([P, NBLK * D_STATE], f32)
        nc.sync.dma_start(alog_sb[:], alog[:])
        negA = const_pool.tile([P, NBLK * D_STATE], f32)
        nc.scalar.activation(negA[:], alog_sb[:], ACT.Exp)
        nc.scalar.mul(negA[:], negA[:], -1.0)
        # diag(D) per block, bf16, for the y-psum init matmul
        diagD = const_pool.tile([P, NBLK, P], bf16)
        for blk in range(NBLK):
            dtmp = const_pool.tile([P, P], f32, tag="dtmp", name="dtmp")
            nc.vector.tensor_scalar_mul(dtmp[:], ident[:],
                                        dvec_sb[:, blk:blk + 1])
            nc.scalar.copy(diagD[:, blk, :], dtmp[:])

        z_sb = z_pool.tile([P, NBLK, L], f32)

        es_A = ExitStack()    # hsT + in_proj weights, spans T+A
        es_xz = ExitStack()   # conv input tiles, spans A..B
        es_u = ExitStack()    # u tiles, spans B..D
        es_wc = ExitStack()   # x_dbl/dt_proj weights, spans C..D
        es_ddu = ExitStack()  # delta+du, spans D..E
        es_E = ExitStack()    # scan transient pools
        es_yg = ExitStack()   # gated y, spans E..F
        es_F = ExitStack()    # out_proj pools
        es_psA = ExitStack()  # matmul psum pools, spans T..D

        try:
            # ===== Phase T+A: transpose hs / in_proj, per time quarter =====
            hsT_pool = es_A.enter_context(tc.tile_pool(name="hsTpool", bufs=1))
            ps_t = es_psA.enter_context(
                tc.tile_pool(name="ps_t", bufs=3, space="PSUM"))
            ps_a = es_psA.enter_context(
                tc.tile_pool(name="ps_a", bufs=4, space="PSUM"))

            hsT = hsT_pool.tile([P, 8, L], f32r, tag="hsT")
            w_r = hsT_pool.tile([P, 8, 1024], f32r, tag="wr")
            with tc.tile_pool(name="wstage", bufs=2) as w_stage:
                for kb in range(8):
                    w_sb = w_stage.tile([P, 1024], f32, tag="w", name=f"w{kb}")
                    nc.sync.dma_start(w_sb[:], wxz[:, kb, :])
                    nc.scalar.copy(w_r[:, kb, :], w_sb[:])

            xz_pool = es_xz.enter_context(
                tc.tile_pool(name="xzpool", bufs=1, side="right"))
            xp = [xz_pool.tile([P, 4 + L], f32, tag=f"xp{b}", name=f"xp{b}")
                  for b in range(NBLK)]
            for blk in range(NBLK):
                nc.vector.memset(xp[blk][:, 0:4], 0.0)

            with tc.tile_pool(name="hsbpool", bufs=2) as hsb_pool:
                for tq in range(NQ):
                    # transpose the 4 token-blocks of this quarter
                    hsb_sb = hsb_pool.tile([P, 4, D_MODEL], f32, tag="hsb",
                                           name=f"hsb{tq}")
                    nc.sync.dma_start(
                        hsb_sb[:],
                        hsb[tq * 512:(tq + 1) * 512, :]
                        .rearrange("(tb p) m -> p tb m", p=P))
                    for kb in range(8):
                        pst = ps_t.tile([P, 512], f32, tag="pst", name="pst")
                        for tb in range(4):
                            nc.tensor.transpose(
                                pst[:, tb * P:(tb + 1) * P],
                                hsb_sb[:, tb, kb * P:(kb + 1) * P],
                                ident[:])
                        nc.vector.tensor_copy(
                            hsT[:, kb, tq * 512:(tq + 1) * 512], pst[:])
                    # in_proj for this quarter
                    for mb in range(8):
                        ps = ps_a.tile([P, 512], f32, tag="psa", name="psa")
                        for kb in range(8):
                            nc.tensor.matmul(
                                ps[:], w_r[:, kb, mb * P:(mb + 1) * P],
                                hsT[:, kb, tq * 512:(tq + 1) * 512],
                                start=(kb == 0), stop=(kb == 7))
                        if mb < 4:
                            nc.scalar.copy(
                                xp[mb][:, 4 + tq * 512: 4 + (tq + 1) * 512],
                                ps[:])
                        else:
                            nc.scalar.copy(
                                z_sb[:, mb - 4, tq * 512:(tq + 1) * 512],
                                ps[:])
            es_A.close()
            if lvl < 2:
                raise _Stop()

            # ============ Phase B: conv + SiLU ============
            u_pool = es_u.enter_context(tc.tile_pool(name="upool", bufs=1))
            u16 = u_pool.tile([P, NBLK, L], bf16)
            with tc.tile_pool(name="cvpool", bufs=2) as cv_pool:
                for blk in range(NBLK):
                    acc = cv_pool.tile([P, L], f32, tag="acc", name="acc")
                    nc.vector.tensor_scalar_mul(
                        acc[:], xp[blk][:, 4:4 + L],
                        convw_sb[:, blk * 4 + 3: blk * 4 + 4])
                    for w in (2, 1, 0):
                        nc.vector.scalar_tensor_tensor(
                            acc[:], xp[blk][:, 1 + w: 1 + w + L],
                            convw_sb[:, blk * 4 + w: blk * 4 + w + 1],
                            acc[:], MULT, ADD)
                    nc.scalar.activation(u16[:, blk, :], acc[:], ACT.Silu,
                                         bias=convb_sb[:, blk:blk + 1])
            es_xz.close()
            if lvl < 3:
                raise _Stop()

            # ============ Phase C: x_dbl partial + AllReduce ============
            wc_pool = es_wc.enter_context(tc.tile_pool(name="wcpool", bufs=1))
            xw_sb = wc_pool.tile([P, NBLK, 96], f32, tag="xw")
            nc.sync.dma_start(xw_sb[:], xprojT[:])
            xw_b = wc_pool.tile([P, NBLK, 96], bf16, tag="xwb")
            nc.scalar.copy(xw_b[:], xw_sb[:])
            with tc.tile_pool(name="xepool", bufs=3) as xe_pool:
                for tq in range(NQ):
                    ps = ps_a.tile([P, 512], f32, tag="psa", name="psa")
                    for kb in range(NBLK):
                        nc.tensor.matmul(ps[0:96, :], xw_b[:, kb, :],
                                         u16[:, kb, tq * 512:(tq + 1) * 512],
                                         start=(kb == 0), stop=(kb == 3))
                    xe = xe_pool.tile([P, 512], bf16, tag="xe", name="xe")
                    nc.scalar.copy(xe[0:96, :], ps[0:96, :])
                    nc.sync.dma_start(
                        xdbl_part[:, tq * 512:(tq + 1) * 512], xe[0:96, :])

            if collectives:
                nc.gpsimd.collective_compute(
                    "AllReduce", ADD, replica_groups=GROUPS,
                    ins=[xdbl_part[:]], outs=[xdbl_sum[:]])
            else:  # timing-sim variant: plain copy stands in for the AR
                nc.sync.dma_start(xdbl_sum[:], xdbl_part[:])

            # ============ Phase D: delta = softplus(dt_proj), du ============
            dtr_sb = wc_pool.tile([DT_RANK, L], bf16, tag="dtr")
            nc.sync.dma_start(dtr_sb[:], xdbl_sum[0:DT_RANK, :])
            dtw_sb = wc_pool.tile([DT_RANK, DC], f32, tag="dtw")
            nc.sync.dma_start(dtw_sb[:], dtprojT[:])
            dtw_b = wc_pool.tile([DT_RANK, DC], bf16, tag="dtwb")
            nc.scalar.copy(dtw_b[:], dtw_sb[:])

            ddu_pool = es_ddu.enter_context(
                tc.tile_pool(name="ddupool", bufs=1, side="right"))
            delta = ddu_pool.tile([P, NBLK, L], bf16)
            du = ddu_pool.tile([P, NBLK, L], bf16)
            # softplus(x+b) = relu(x+b) + ln(1 + exp(-|x+b|)); the deployed
            # Softplus table is unavailable, so compose it. Batch the Exp
            # and Ln sub-steps per block so the Exp<->Ln table sets load
            # once per batch instead of per chunk.
            with tc.tile_pool(name="sppool", bufs=1) as sp_pool:
                for blk in range(NBLK):
                    pss, tes = [], []
                    for tq in range(NQ):
                        ps = ps_a.tile([P, 512], f32, tag="psa", name="psa")
                        nc.tensor.matmul(
                            ps[:], dtw_b[:, blk * P:(blk + 1) * P],
                            dtr_sb[:, tq * 512:(tq + 1) * 512])
                        pss.append(ps)
                        ta = sp_pool.tile([P, 512], f32, tag="ta", name="ta")
                        nc.scalar.activation(ta[:], ps[:], ACT.Abs,
                                             bias=dtb_sb[:, blk:blk + 1])
                        te = sp_pool.tile([P, 512], f32, tag=f"te{tq}",
                                          name="te")
                        nc.scalar.activation(te[:], ta[:], ACT.Exp, scale=-1.0)
                        tes.append(te)
                    for tq in range(NQ):
                        dchunk = delta[:, blk, tq * 512:(tq + 1) * 512]
                        tl = sp_pool.tile([P, 512], f32, tag="tl", name="tl")
                        nc.scalar.activation(tl[:], tes[tq][:], ACT.Ln,
                                             bias=1.0)
                        tr = sp_pool.tile([P, 512], f32, tag="tr", name="tr")
                        nc.scalar.activation(tr[:], pss[tq][:], ACT.Relu,
                                             bias=dtb_sb[:, blk:blk + 1])
                        nc.vector.tensor_tensor(dchunk, tr[:], tl[:],
                                                ADD)
                    nc.vector.tensor_tensor(du[:, blk, :], delta[:, blk, :],
                                            u16[:, blk, :], MULT)
                    nc.sync.dma_start(uspill[blk * P:(blk + 1) * P, :],
                                      u16[:, blk, :])
            es_wc.close()
            es_u.close()
            es_psA.close()
            if lvl < 4:
                raise _Stop()

            # ============ Phase E: selective scan (two halves) ============
            yg_pool = es_yg.enter_context(tc.tile_pool(name="ygpool", bufs=1))
            bc_pool = es_E.enter_context(tc.tile_pool(name="bcpool", bufs=2))
            sc_pool = es_E.enter_context(tc.tile_pool(name="scpool", bufs=3))
            s_pool = es_E.enter_context(tc.tile_pool(name="spool", bufs=2))
            g_pool = es_E.enter_context(tc.tile_pool(name="gpool", bufs=1))
            carry_pool = es_E.enter_context(tc.tile_pool(name="carrypool",
                                                         bufs=1))
            ps_y = es_E.enter_context(
                tc.tile_pool(name="ps_y", bufs=4, space="PSUM"))
            yg = yg_pool.tile([P, NBLK, L], f32r)
            carry = carry_pool.tile([P, NBLK * D_STATE], f32)

            for half in range(2):
                h0 = half * H
                y_ps = {}
                for blk in range(NBLK):
                    y_ps[blk] = ps_y.tile([P, H], f32, tag="yps",
                                          name=f"yps{blk}")
                    uc = g_pool.tile([P, H], bf16, tag="uc", name="uc")
                    nc.sync.dma_start(
                        uc[:], uspill[blk * P:(blk + 1) * P, h0:h0 + H])
                    for c in range(2):
                        nc.tensor.matmul(
                            y_ps[blk][:, c * 512:(c + 1) * 512],
                            diagD[:, blk, :],
                            uc[:, c * 512:(c + 1) * 512],
                            start=True, stop=False)
                for n in range(D_STATE):
                    # B_n and C_n rows are interleaved host-side: one
                    # stride-0 partition-broadcast DMA fetches both.
                    bc = bc_pool.tile([P, 2, H], bf16, tag="bc", name="bc")
                    nc.sync.dma_start(
                        bc[:],
                        xdbl_sum[DT_RANK + 2 * n: DT_RANK + 2 * n + 2,
                                 h0:h0 + H]
                        .partition_broadcast(P))
                    for blk in range(NBLK):
                        idx = blk * D_STATE + n
                        dchunk = delta[:, blk, h0:h0 + H]
                        # dA stays fp32: its error compounds over the decay
                        # length, unlike every other bf16-rounded term.
                        dA = sc_pool.tile([P, H], f32, tag="dA", name="dA")
                        nc.scalar.activation(
                            dA[:], dchunk, ACT.Exp,
                            scale=negA[:, idx:idx + 1])
                        dBu = sc_pool.tile([P, H], bf16, tag="dBu",
                                           name="dBu")
                        # GpSimd takes most dBu multiplies (it cannot run
                        # scans or stt: codegen rejects TensorScalarPtr on
                        # Pool); DVE takes the rest to balance.
                        if n == 5:
                            nc.vector.tensor_tensor(
                                dBu[:], du[:, blk, h0:h0 + H], bc[:, 0, :],
                                MULT)
                        else:
                            nc.gpsimd.tensor_tensor(
                                dBu[:], du[:, blk, h0:h0 + H], bc[:, 0, :],
                                MULT)
                        s = s_pool.tile([P, H], bf16, tag="s", name="s")
                        nc.vector.tensor_tensor_scan(
                            s[:], dA[:], dBu[:],
                            0.0 if half == 0 else carry[:, idx:idx + 1],
                            MULT, ADD)
                        if half == 0:
                            nc.scalar.copy(carry[:, idx:idx + 1],
                                           s[:, H - 1:H])
                        sC = s_pool.tile([P, H], bf16, tag="sC", name="sC")
                        nc.vector.tensor_tensor(sC[:], s[:], bc[:, 1, :],
                                                MULT)
                        for c in range(2):
                            nc.tensor.matmul(
                                y_ps[blk][:, c * 512:(c + 1) * 512],
                                ident16[:],
                                sC[:, c * 512:(c + 1) * 512],
                                start=False, stop=(n == D_STATE - 1))
                # gate: yg = y * z * (tanh(z/2) + 1); the 0.5 factor is
                # folded into the out_proj weights host-side.
                for blk in range(NBLK):
                    zchunk = z_sb[:, blk, h0:h0 + H]
                    th = g_pool.tile([P, H], f32, tag="th", name="th")
                    nc.scalar.activation(th[:], zchunk, ACT.Tanh, scale=0.5)
                    t1 = g_pool.tile([P, H], f32, tag="t1", name="t1")
                    nc.vector.scalar_tensor_tensor(
                        t1[:], th[:], ones_col[:], zchunk, ADD, MULT)
                    nc.vector.tensor_tensor(
                        yg[:, blk, h0:h0 + H], y_ps[blk][:], t1[:], MULT)
            es_ddu.close()
            es_E.close()
            if lvl < 5:
                raise _Stop()

            # ===== Phase F: out_proj partial + ReduceScatter =====
            ow_pool = es_F.enter_context(tc.tile_pool(name="owpool", bufs=1))
            oe_pool = es_F.enter_context(tc.tile_pool(name="oepool", bufs=3))
            ps_o = es_F.enter_context(
                tc.tile_pool(name="ps_o", bufs=4, space="PSUM"))

            ow_r = ow_pool.tile([P, NBLK, 1024], f32r, tag="owr")
            with tc.tile_pool(name="owstage", bufs=2) as ow_stage:
                for kb in range(NBLK):
                    ow_sb = ow_stage.tile([P, 1024], f32, tag="ow",
                                          name=f"ow{kb}")
                    nc.sync.dma_start(ow_sb[:], outw[:, kb, :])
                    nc.scalar.copy(ow_r[:, kb, :], ow_sb[:])

            for mb in range(8):
                ot = oe_pool.tile([P, L], f32, tag="ot", name="ot")
                for tq in range(NQ):
                    po = ps_o.tile([P, 512], f32, tag="po", name="po")
                    for kb in range(NBLK):
                        nc.tensor.matmul(
                            po[:], ow_r[:, kb, mb * P:(mb + 1) * P],
                            yg[:, kb, tq * 512:(tq + 1) * 512],
                            start=(kb == 0), stop=(kb == 3))
                    nc.scalar.copy(ot[:, tq * 512:(tq + 1) * 512], po[:])
                nc.sync.dma_start(outpart[mb * P:(mb + 1) * P, :], ot[:])

            if collectives:
                nc.gpsimd.collective_compute(
                    "ReduceScatter", ADD, replica_groups=GROUPS,
                    ins=[outpart[:]], outs=[outrs[:]])
            else:
                nc.sync.dma_start(outrs[:], outpart[0:D_MODEL // 4, :])
            nc.sync.dma_start(out_shard[:], outrs[:])
            es_F.close()
            es_yg.close()
        except _Stop:
            pass
        finally:
            for es in (es_A, es_xz, es_u, es_wc, es_ddu, es_E,
                       es_F, es_yg, es_psA):
                es.close()

    nc.compile()
    return nc


def _shard_inputs(inputs):
    hs = np.ascontiguousarray(np.asarray(inputs["hidden_states"], np.float32))
    win = np.asarray(inputs["in_proj_w"], np.float32)
    convw = np.asarray(inputs["conv_w"], np.float32)
    convb = np.asarray(inputs["conv_b"], np.float32)
    xproj = np.asarray(inputs["x_proj_w"], np.float32)
    dtw = np.asarray(inputs["dt_proj_w"], np.float32)
    dtb = np.asarray(inputs["dt_proj_b"], np.float32)
    alog = np.asarray(inputs["A_log"], np.float32)
    dv = np.asarray(inputs["D"], np.float32)
    outw = np.asarray(inputs["out_proj_w"], np.float32)

    # permute x_proj rows so B_n/C_n come out interleaved: one broadcast DMA
    # per state-dim fetches both rows.
    perm = list(range(DT_RANK))
    for n in range(D_STATE):
        perm.append(DT_RANK + n)            # B_n -> row 64+2n
        perm.append(DT_RANK + D_STATE + n)  # C_n -> row 65+2n
    xproj_p = xproj[perm]

    in_maps = []
    for c in range(NCORES):
        b, k = c // 4, c % 4
        cs, ce = k * DC, (k + 1) * DC
        wxT = win[cs:ce].T            # [1024, 512]
        wzT = win[D_INNER + cs:D_INNER + ce].T
        wcat = np.concatenate([wxT, wzT], axis=1)  # [1024, 1024]
        wxz = np.ascontiguousarray(
            wcat.reshape(8, P, 1024).transpose(1, 0, 2))  # [128, 8, 1024]
        in_maps.append({
            "hsb": hs[b],
            "wxz": wxz,
            "convw": np.ascontiguousarray(
                convw[cs:ce].reshape(NBLK, P, D_CONV).transpose(1, 0, 2)
                .reshape(P, NBLK * D_CONV)),
            "convb": np.ascontiguousarray(convb[cs:ce].reshape(NBLK, P).T),
            "xprojT": np.ascontiguousarray(
                xproj_p[:, cs:ce].T.reshape(NBLK, P, 96).transpose(1, 0, 2)),
            "dtprojT": np.ascontiguousarray(dtw[cs:ce].T),
            "dtb": np.ascontiguousarray(dtb[cs:ce].reshape(NBLK, P).T),
            "alog": np.ascontiguousarray(
                alog[cs:ce].reshape(NBLK, P, D_STATE).transpose(1, 0, 2)
                .reshape(P, NBLK * D_STATE)),
            "dvec": np.ascontiguousarray(dv[cs:ce].reshape(NBLK, P).T),
            # 0.5 * W_out^T: the tanh-form gate leaves a factor 2
            "outw": np.ascontiguousarray(
                (0.5 * outw[:, cs:ce]).T.reshape(NBLK, P, 1024)
                .transpose(1, 0, 2)),
        })
    return in_maps


def kernel(**inputs):
    global LAST_RESULTS
    from concourse.bass_utils import run_bass_kernel_spmd

    if "prog" not in _prog_cache:
        _prog_cache["prog"] = _build_program()
    nc = _prog_cache["prog"]

    in_maps = _shard_inputs(inputs)
    res = run_bass_kernel_spmd(nc, in_maps, list(range(NCORES)),
                               trace=TRACE)
    LAST_RESULTS = res

    out = np.empty((BATCH, L, D_MODEL), np.float32)
    for g in range(BATCH):
        rows = np.concatenate(
            [res.results[g * 4 + i]["out_shard"] for i in range(4)], axis=0)
        out[g] = rows.T
    return out

